# revision 15
# baseline (speedup 1.0000x reference)
"""Binarized MLP forward (BinaryConnect, training-mode BatchNorm) on 8 TRN2 cores.

Strategy: data-parallel over the batch (16384 -> 8 x 2048), weights replicated.
Activations kept TRANSPOSED on device ([features, batch]); matmuls in f32r
(exact fp32; issue rate ~1 row/cycle with 2-pass pipelining).

Schedule: per-feature batch stats are all-reduced in 5 (L1) / 4 (L2,L3) small
groups, launched as soon as each j-tile's stats are complete, so the collective
round trips hide under the next ~8-45us of matmul work:
 - L1 computes b0/b1 as k-outer sweeps (chases the streaming x DMA), then
   (j,b2),(j,b3) pairs j-outer so early j-tiles finish stats early.
 - L2/L3 contract k=0..5 first for j0/j1 (whose k6,k7 inputs are the previous
   layer's last-finished stats group) and patch the k6,k7 contribution in via
   PSUM + DVE add after the j2 block, by which point the late group's BN apply
   has long completed. All other j-tiles contract k=0..7 inline.
 - The head holds its 4 PSUM banks open over k0..6 and appends k7 when the
   last group's apply lands.
BN scale/shift are computed on VectorE only (bit-trick + Newton rsqrt, no
ScalarE sqrt), so ScalarE runs a single activation table set
(sigmoid_and_others: sign/relu/sigmoid) loaded once at t=0.
"""
import os
import numpy as np

import concourse.bass as bass
import concourse.bacc as bacc
import concourse.tile as tile
from concourse.tile_rust import add_dep_helper
import concourse.mybir as mybir
from concourse.bass_utils import run_bass_kernel_spmd

N_CORES = 8
B_TOT = 16384
BPC = B_TOT // N_CORES  # 2048 batch rows per core
NB = BPC // 512  # 4 free-dim tiles of 512
D_IN, H, D_OUT = 784, 1024, 10
D_IN_PAD = 896  # pad 784 -> 7 full k-tiles of 128
KT1 = D_IN_PAD // 128
NJ = H // 128  # 8 feature tiles per hidden layer
BN_EPS = 1e-5
RSQRT_MAGIC = 0x5F3759DF

f32 = mybir.dt.float32
f32r = mybir.dt.float32r
i32 = mybir.dt.int32
bf16 = mybir.dt.bfloat16
AF = mybir.ActivationFunctionType
ALU = mybir.AluOpType

# t_vec scratch layout (free-dim float offsets, indexed by j 0..7)
V_S = 0       # BN scale per feature
V_T = 8       # BN shift per feature
V_M = 16      # mean
V_E2 = 24     # E[x^2]
V_VU = 32     # var + eps
V_R = 40      # rsqrt iterate
V_T1 = 48     # newton scratch
V_TMP = 56    # misc scratch
V_TMP2 = 64

# stats groups: list of (j_lo, j_hi) per layer
GROUPS_L1 = [(0, 2), (2, 4), (4, 6), (6, 7), (7, 8)]
GROUPS_H = [(0, 4), (4, 6), (6, 7), (7, 8)]


def build(nc):
    xT = nc.dram_tensor("xT", [KT1, NB, 128, 512], f32r, kind="ExternalInput")
    w1t = nc.dram_tensor("w1t", [D_IN_PAD, H], bf16, kind="ExternalInput")
    w2t = nc.dram_tensor("w2t", [H, H], bf16, kind="ExternalInput")
    w3t = nc.dram_tensor("w3t", [H, H], bf16, kind="ExternalInput")
    w4t = nc.dram_tensor("w4t", [H, D_OUT], bf16, kind="ExternalInput")
    gbp = nc.dram_tensor("gbp", [128, 6, 8], f32, kind="ExternalInput")
    outT = nc.dram_tensor("outT", [D_OUT, BPC], f32, kind="ExternalOutput")

    rg = [list(range(N_CORES))]

    with tile.TileContext(nc) as tc:
        with (
            tc.tile_pool(name="hp", bufs=2) as hpool,
            tc.tile_pool(name="wp", bufs=2) as wpool,
            tc.tile_pool(name="w4p", bufs=1) as w4pool,
            tc.tile_pool(name="stg", bufs=8) as stgpool,
            tc.tile_pool(name="outp", bufs=2) as outpool,
            tc.tile_pool(name="msc", bufs=1) as mpool,
            tc.tile_pool(name="ps", bufs=8, space="PSUM") as pspool,
            tc.tile_pool(name="dram", bufs=1, space="DRAM") as dpool,
        ):
            t_stats = mpool.tile([128, 192], f32, name="t_stats")
            t_part = mpool.tile([128, 16], f32, name="t_part")
            t_gst = mpool.tile([128, 16], f32, name="t_gst")
            t_vec = mpool.tile([128, 72], f32, name="t_vec")
            t_gb = mpool.tile([128, 48], f32, name="t_gb")
            t_const = mpool.tile([128, 48], f32, name="t_const")

            # single act-table load: force the sigmoid set (contains
            # sign/relu/sigmoid) before any other ScalarE op
            nc.scalar.activation(
                t_vec[:, V_TMP : V_TMP + 1], t_vec[:, V_TMP : V_TMP + 1], AF.Sigmoid
            )
            C_OFF = {"eighth": 0, "eps": 8, "one": 16, "magic": 24, "nhalf": 32, "3half": 40}
            nc.vector.memset(t_const[:, 0:8], 1.0 / N_CORES)
            nc.vector.memset(t_const[:, 8:16], BN_EPS)
            nc.vector.memset(t_const[:, 16:24].bitcast(i32), 1)
            nc.vector.memset(t_const[:, 24:32].bitcast(i32), RSQRT_MAGIC)
            nc.vector.memset(t_const[:, 32:40], -0.5)
            nc.vector.memset(t_const[:, 40:48], 1.5)

            # --- warmup collective: absorb first-call ncfw/algorithm cost.
            with nc.named_scope("warmup_ar"):
                win = dpool.tile([128, 2], f32, name="warm_in")
                wout = dpool.tile([128, 2], f32, name="warm_out", addr_space="Shared")
                nc.gpsimd.collective_compute(
                    "AllReduce", ALU.add, replica_groups=rg,
                    ins=[win[:].opt()], outs=[wout[:].opt()],
                )

            # ---------------- input DMAs + weight sign prep ----------------
            xT_t = hpool.tile([128, KT1, NB, 512], f32r, name="xT_t", tag="h")
            W1s = wpool.tile([128, KT1, H], f32r, name="W1s", tag="w")
            W2s = wpool.tile([128, NJ, H], f32r, name="W2s", tag="w")
            W3s = wpool.tile([128, NJ, H], f32r, name="W3s", tag="w")
            W4s = w4pool.tile([128, NJ, D_OUT], f32r, name="W4s")

            def emit_w_dmas(wt_dram, nkt, tag):
                stgs = []
                for k in range(nkt):
                    for hh in range(2):
                        stg = stgpool.tile(
                            [128, H // 2], bf16, name=f"stg_{tag}_{k}_{hh}", tag="stg"
                        )
                        nc.sync.dma_start(
                            stg[:],
                            wt_dram[k * 128 : (k + 1) * 128, hh * 512 : (hh + 1) * 512],
                        )
                        stgs.append((k, hh, stg))
                return stgs

            def emit_w_signs(Wtile, stgs, after=None):
                first = [after]
                for k, hh, stg in stgs:
                    si = nc.scalar.activation(
                        Wtile[:, k, hh * 512 : (hh + 1) * 512], stg[:], AF.Sign
                    )
                    if first[0] is not None:
                        add_dep_helper(
                            si.ins, first[0], False,
                            "keep boundary ACT ops ahead of weight signs",
                        )
                        first[0] = None

            # w1 + x(b0) interleaved, then xb1, w2, xb2, xb3, gbp+w4, w3
            with nc.named_scope("w1prep"):
                # interleave per k: two w1 halves then the x tile
                for k in range(KT1):
                    for hh in range(2):
                        stg = stgpool.tile(
                            [128, H // 2], bf16, name=f"stg_w1_{k}_{hh}", tag="stg"
                        )
                        nc.sync.dma_start(
                            stg[:],
                            w1t[k * 128 : (k + 1) * 128, hh * 512 : (hh + 1) * 512],
                        )
                        nc.scalar.activation(
                            W1s[:, k, hh * 512 : (hh + 1) * 512], stg[:], AF.Sign
                        )
                    nc.sync.dma_start(xT_t[:, k, 0], xT[k, 0])
            with nc.named_scope("xload1"):
                for k in range(KT1):
                    nc.sync.dma_start(xT_t[:, k, 1], xT[k, 1])
            with nc.named_scope("w2load"):
                w2_stgs = emit_w_dmas(w2t, NJ, "w2")
            with nc.named_scope("xload23"):
                for b in (2, 3):
                    for k in range(KT1):
                        nc.sync.dma_start(xT_t[:, k, b], xT[k, b])
            nc.sync.dma_start(t_gb[:], gbp[:].rearrange("p a b -> p (a b)"))
            with nc.named_scope("w4prep"):
                stg4 = stgpool.tile([128, NJ, D_OUT], bf16, name="stg_w4", tag="stg")
                nc.sync.dma_start(
                    stg4[:], w4t[:].rearrange("(kt p) c -> p kt c", p=128)
                )
                nc.scalar.activation(
                    W4s[:].rearrange("p a b -> p (a b)"),
                    stg4[:].rearrange("p a b -> p (a b)"),
                    AF.Sign,
                )
            with nc.named_scope("w3load"):
                w3_stgs = emit_w_dmas(w3t, NJ, "w3")
            # w2 signs early (ScalarE idle during L1 until first applies)
            with nc.named_scope("w2sign"):
                emit_w_signs(W2s, w2_stgs)

            h1 = hpool.tile([128, NJ, NB, 512], f32r, name="h1", tag="h")
            h2 = hpool.tile([128, NJ, NB, 512], f32r, name="h2", tag="h")
            h3 = hpool.tile([128, NJ, NB, 512], f32r, name="h3", tag="h")

            # w3 signs are pinned after L1's last apply group so the scheduler
            # cannot hoist the sign burst into the L1->L2 boundary ACT window
            l1_last_apply = [None]

            # ---------------- helpers ----------------
            def drain_stats(out_h, j, b, with_stats=True):
                so = j * 24 + b * 6
                nc.vector.tensor_copy(out_h[:, j, b], cur_acc[(j, b)][:])
                if with_stats:
                    nc.vector.bn_stats(
                        t_stats[:, so : so + 6], out_h[:, j, b].bitcast(f32)
                    )

            cur_acc = {}

            def aggr_j(j):
                po = j * 2
                nc.vector.bn_aggr(
                    t_part[:, po : po + 2], t_stats[:, j * 24 : j * 24 + 24]
                )
                # rsqrt seed from the LOCAL variance (no collective wait):
                # rr[j] = bitcast(MAGIC - ((var_loc + eps)_int >> 1)); the
                # Newton steps in st_group then converge on the global var.
                tj = t_vec[:, V_T1 + j : V_T1 + j + 1]
                rj = t_vec[:, V_R + j : V_R + j + 1]
                nc.vector.tensor_scalar(
                    tj, t_part[:, po + 1 : po + 2], BN_EPS, None, op0=ALU.add
                )
                nc.vector.tensor_scalar(
                    tj.bitcast(i32), tj.bitcast(i32), 1, None,
                    op0=ALU.logical_shift_right,
                )
                nc.vector.tensor_tensor(
                    rj.bitcast(i32),
                    t_const[:, C_OFF["magic"] + j : C_OFF["magic"] + j + 1].bitcast(i32),
                    tj.bitcast(i32),
                    op=ALU.subtract,
                )
                # fold mean^2 into E2 slot: part[1] += part[0]^2
                nc.vector.tensor_tensor(
                    t_vec[:, V_TMP + j : V_TMP + j + 1],
                    t_part[:, po : po + 1],
                    t_part[:, po : po + 1],
                    op=ALU.mult,
                )
                nc.vector.tensor_tensor(
                    t_part[:, po + 1 : po + 2],
                    t_vec[:, V_TMP + j : V_TMP + j + 1],
                    t_part[:, po + 1 : po + 2],
                    op=ALU.add,
                )

            def collective_group(li, j_lo, j_hi):
                n = (j_hi - j_lo) * 2
                with nc.named_scope(f"L{li}_ar{j_lo}"):
                    cin = dpool.tile([128, n], f32, name=f"cin{li}_{j_lo}")
                    cout = dpool.tile(
                        [128, n], f32, name=f"cout{li}_{j_lo}", addr_space="Shared"
                    )
                    nc.gpsimd.dma_start(cin[:], t_part[:, j_lo * 2 : j_hi * 2])
                    nc.gpsimd.collective_compute(
                        "AllReduce", ALU.add, replica_groups=rg,
                        ins=[cin[:].opt()], outs=[cout[:].opt()],
                    )
                    nc.gpsimd.dma_start(t_gst[:, j_lo * 2 : j_hi * 2], cout[:])

            def st_group(li, j_lo, j_hi):
                """s = g*rsqrt(v+eps), t = b - m*s — entirely on GpSimd
                (tensor_tensor against constant tiles only, so the ops ride
                the same Pool queue as the collective and can never block
                VectorE drains). rsqrt via bit-trick init + 3 Newton steps."""
                with nc.named_scope(f"L{li}_st{j_lo}"):
                    gview = t_gst[:, j_lo * 2 : j_hi * 2].rearrange(
                        "p (j c) -> p j c", c=2
                    )
                    mm = t_vec[:, V_M + j_lo : V_M + j_hi]
                    e2 = t_vec[:, V_E2 + j_lo : V_E2 + j_hi]
                    vu = t_vec[:, V_VU + j_lo : V_VU + j_hi]
                    rr = t_vec[:, V_R + j_lo : V_R + j_hi]
                    t1 = t_vec[:, V_T1 + j_lo : V_T1 + j_hi]
                    tp2 = t_vec[:, V_TMP2 + j_lo : V_TMP2 + j_hi]
                    sv = t_vec[:, V_S + j_lo : V_S + j_hi]
                    tv = t_vec[:, V_T + j_lo : V_T + j_hi]
                    cs = lambda name: t_const[:, C_OFF[name] + j_lo : C_OFF[name] + j_hi]
                    g_sl = t_gb[:, (li - 1) * 16 + j_lo : (li - 1) * 16 + j_hi]
                    b_sl = t_gb[:, (li - 1) * 16 + 8 + j_lo : (li - 1) * 16 + 8 + j_hi]
                    tt = nc.gpsimd.tensor_tensor
                    tt(mm, gview[:, :, 0], cs("eighth"), op=ALU.mult)
                    tt(e2, gview[:, :, 1], cs("eighth"), op=ALU.mult)
                    tt(tp2, mm, mm, op=ALU.mult)
                    tt(vu, e2, tp2, op=ALU.subtract)
                    tt(vu, vu, cs("eps"), op=ALU.add)
                    # rr was seeded in aggr_j from the local variance
                    for _ in range(3):
                        tt(t1, rr, rr, op=ALU.mult)
                        tt(t1, t1, vu, op=ALU.mult)
                        tt(t1, t1, cs("nhalf"), op=ALU.mult)
                        tt(t1, t1, cs("3half"), op=ALU.add)
                        tt(rr, rr, t1, op=ALU.mult)
                    tt(sv, g_sl, rr, op=ALU.mult)
                    tt(tp2, mm, sv, op=ALU.mult)
                    last = tt(tv, b_sl, tp2, op=ALU.subtract)
                    return last

            def apply_group(li, out_h, j_lo, j_hi):
                """BN scale/shift + ReLU in place on ScalarE, b-outer."""
                last = None
                with nc.named_scope(f"L{li}_apply{j_lo}"):
                    for b in range(NB):
                        for j in range(j_lo, j_hi):
                            last = nc.scalar.activation(
                                out_h[:, j, b],
                                out_h[:, j, b].bitcast(f32),
                                AF.Relu,
                                bias=t_vec[:, V_T + j : V_T + j + 1],
                                scale=t_vec[:, V_S + j : V_S + j + 1],
                            )
                return last

            # ---------------- layer 1: x -> h1 ----------------
            with nc.named_scope("L1_sweepA"):
                # b0, b1 as k-outer sweeps (chases the x DMA stream)
                for b in (0, 1):
                    accs = [
                        pspool.tile([128, 512], f32, name=f"ps1_s{b}_j{j}", tag="ps")
                        for j in range(NJ)
                    ]
                    for k in range(KT1):
                        for j in range(NJ):
                            nc.tensor.matmul(
                                accs[j][:],
                                W1s[:, k, j * 128 : (j + 1) * 128],
                                xT_t[:, k, b],
                                start=(k == 0),
                                stop=(k == KT1 - 1),
                            )
                    for j in range(NJ):
                        cur_acc[(j, b)] = accs[j]
                        drain_stats(h1, j, b)
            with nc.named_scope("L1_phaseB"):
                gi = 0
                for j in range(NJ):
                    for b in (2, 3):
                        acc = pspool.tile(
                            [128, 512], f32, name=f"ps1_p_j{j}_b{b}", tag="ps"
                        )
                        for k in range(KT1):
                            nc.tensor.matmul(
                                acc[:],
                                W1s[:, k, j * 128 : (j + 1) * 128],
                                xT_t[:, k, b],
                                start=(k == 0),
                                stop=(k == KT1 - 1),
                            )
                        cur_acc[(j, b)] = acc
                        drain_stats(h1, j, b)
                    aggr_j(j)
                    if gi < len(GROUPS_L1) and j == GROUPS_L1[gi][1] - 1:
                        collective_group(1, *GROUPS_L1[gi])
                        st_group(1, *GROUPS_L1[gi])
                        la = apply_group(1, h1, *GROUPS_L1[gi])
                        l1_last_apply[0] = la.ins
                        gi += 1

            def hidden_layer(li, Wcur, rhs, out_h, post_j2=None):
                """L2/L3: j0/j1 contract k0..5 with a k6,k7 fixup after j2."""
                gi = [0]
                groups = GROUPS_H

                def maybe_group(j):
                    if gi[0] < len(groups) and j == groups[gi[0]][1] - 1:
                        g = groups[gi[0]]
                        collective_group(li, *g)
                        st_group(li, *g)
                        apply_group(li, out_h, *g)
                        gi[0] += 1

                def jblock(j, klo, khi):
                    with nc.named_scope(f"L{li}_j{j}"):
                        for b in range(NB):
                            acc = pspool.tile(
                                [128, 512], f32, name=f"ps{li}_j{j}_b{b}", tag="ps"
                            )
                            for k in range(klo, khi):
                                nc.tensor.matmul(
                                    acc[:],
                                    Wcur[:, k, j * 128 : (j + 1) * 128],
                                    rhs[:, k, b],
                                    start=(k == klo),
                                    stop=(k == khi - 1),
                                )
                            cur_acc[(j, b)] = acc
                            drain_stats(out_h, j, b, with_stats=(klo == 0 and khi == NJ))

                jblock(0, 0, NJ - 2)
                jblock(1, 0, NJ - 2)
                jblock(2, 0, NJ)
                aggr_j(2)
                if post_j2 is not None:
                    post_j2()
                # fixup: k6,k7 for j0,j1 via PSUM + DVE add
                with nc.named_scope(f"L{li}_fix"):
                    for j in (0, 1):
                        for b in range(NB):
                            facc = pspool.tile(
                                [128, 512], f32, name=f"psf{li}_j{j}_b{b}", tag="ps"
                            )
                            for k in (NJ - 2, NJ - 1):
                                nc.tensor.matmul(
                                    facc[:],
                                    Wcur[:, k, j * 128 : (j + 1) * 128],
                                    rhs[:, k, b],
                                    start=(k == NJ - 2),
                                    stop=(k == NJ - 1),
                                )
                            nc.vector.tensor_tensor(
                                out_h[:, j, b],
                                out_h[:, j, b].bitcast(f32),
                                facc[:],
                                op=ALU.add,
                            )
                            so = j * 24 + b * 6
                            nc.vector.bn_stats(
                                t_stats[:, so : so + 6], out_h[:, j, b].bitcast(f32)
                            )
                        aggr_j(j)
                for j in range(3, NJ):
                    jblock(j, 0, NJ)
                    aggr_j(j)
                    maybe_group(j)
                # j==3 triggers the {0..3} group (after fixup aggrs)
                # handled by maybe_group inside the loop; groups[0]=(0,4)

            def post_j2_w3signs():
                with nc.named_scope("w3sign"):
                    emit_w_signs(W3s, w3_stgs, after=l1_last_apply[0])

            hidden_layer(2, W2s, h1, h2, post_j2=post_j2_w3signs)
            hidden_layer(3, W3s, h2, h3)

            # ---------------- head: h3 -> sigmoid out ----------------
            with nc.named_scope("L4"):
                haccs = []
                for b in range(NB):
                    acc = pspool.tile([D_OUT, 512], f32, name=f"ps4_b{b}", tag="ps")
                    for k in range(NJ - 1):
                        nc.tensor.matmul(
                            acc[:], W4s[:, k], h3[:, k, b],
                            start=(k == 0), stop=False,
                        )
                    haccs.append(acc)
                for b in range(NB):
                    nc.tensor.matmul(
                        haccs[b][:], W4s[:, NJ - 1], h3[:, NJ - 1, b],
                        start=False, stop=True,
                    )
                    osb = outpool.tile([D_OUT, 512], f32, name=f"osb{b}", tag="osb")
                    nc.scalar.activation(osb[:], haccs[b][:], AF.Sigmoid)
                    nc.sync.dma_start(outT[:, b * 512 : (b + 1) * 512], osb[:])

    nc.compile()
    return nc


_NC = None
_LAST_RESULTS = None


def _get_nc():
    global _NC
    if _NC is None:
        nc = bacc.Bacc(
            "TRN2", target_bir_lowering=False, debug=False, num_devices=N_CORES
        )
        build(nc)
        _NC = nc
    return _NC


def kernel(**inputs):
    x = np.ascontiguousarray(inputs["x"], dtype=np.float32)
    w1 = np.asarray(inputs["w1"], dtype=np.float32)
    w2 = np.asarray(inputs["w2"], dtype=np.float32)
    w3 = np.asarray(inputs["w3"], dtype=np.float32)
    w4 = np.asarray(inputs["w4"], dtype=np.float32)
    gb = np.stack(
        [
            np.asarray(inputs[n], dtype=np.float32)
            for n in ("g1", "b1", "g2", "b2", "g3", "b3")
        ]
    )  # [6, 1024]

    import ml_dtypes

    bf = ml_dtypes.bfloat16
    w1tp = np.zeros((D_IN_PAD, H), bf)
    w1tp[:D_IN] = w1.T.astype(bf)
    w2tp = np.ascontiguousarray(w2.T.astype(bf))
    w3tp = np.ascontiguousarray(w3.T.astype(bf))
    w4tp = np.ascontiguousarray(w4.T.astype(bf))
    gbp = np.ascontiguousarray(gb.reshape(6, 8, 128).transpose(2, 0, 1))  # [128,6,8]

    nc = _get_nc()
    in_maps = []
    for c in range(N_CORES):
        xs = np.zeros((D_IN_PAD, BPC), np.float32)
        xs[:D_IN] = x[c * BPC : (c + 1) * BPC].T
        xs = np.ascontiguousarray(
            xs.reshape(KT1, 128, NB, 512).transpose(0, 2, 1, 3)
        )
        in_maps.append(
            {"xT": xs, "w1t": w1tp, "w2t": w2tp, "w3t": w3tp, "w4t": w4tp, "gbp": gbp}
        )

    last_err = None
    for _attempt in range(3):
        try:
            res = run_bass_kernel_spmd(nc, in_maps, core_ids=list(range(N_CORES)))
            break
        except Exception as e:  # transient NRT_EXEC_UNIT_UNRECOVERABLE etc.
            last_err = e
    else:
        raise last_err
    global _LAST_RESULTS
    _LAST_RESULTS = res
    out = np.empty((B_TOT, D_OUT), dtype=np.float32)
    for c in range(N_CORES):
        out[c * BPC : (c + 1) * BPC] = res.results[c]["outT"].T
    return out


# revision 23
# speedup vs baseline: 1.1492x; 1.1492x over previous
"""Binarized MLP forward (BinaryConnect, training-mode BatchNorm) on 8 TRN2 cores.

Strategy: data-parallel over the batch (16384 -> 8 x 2048), weights replicated.
Activations kept TRANSPOSED on device ([features, batch]); matmuls in f32r
(exact fp32; issue rate ~1 row/cycle with 2-pass pipelining).

Schedule: per-feature batch stats are all-reduced in 5 (L1) / 4 (L2,L3) small
groups, launched as soon as each j-tile's stats are complete, so the collective
round trips hide under the next ~8-45us of matmul work:
 - L1 computes b0/b1 as k-outer sweeps (chases the streaming x DMA), then
   (j,b2),(j,b3) pairs j-outer so early j-tiles finish stats early.
 - L2/L3 contract k=0..5 first for j0/j1 (whose k6,k7 inputs are the previous
   layer's last-finished stats group) and patch the k6,k7 contribution in via
   PSUM + DVE add after the j2 block, by which point the late group's BN apply
   has long completed. All other j-tiles contract k=0..7 inline.
 - The head holds its 4 PSUM banks open over k0..6 and appends k7 when the
   last group's apply lands.
BN scale/shift are computed on VectorE only (bit-trick + Newton rsqrt, no
ScalarE sqrt), so ScalarE runs a single activation table set
(sigmoid_and_others: sign/relu/sigmoid) loaded once at t=0.
"""
import os
import numpy as np

import concourse.bass as bass
import concourse.bacc as bacc
import concourse.tile as tile
from concourse.tile_rust import add_dep_helper
import concourse.mybir as mybir
from concourse.bass_utils import run_bass_kernel_spmd

N_CORES = 8
B_TOT = 16384
BPC = B_TOT // N_CORES  # 2048 batch rows per core
NB = BPC // 512  # 4 free-dim tiles of 512
D_IN, H, D_OUT = 784, 1024, 10
D_IN_PAD = 896  # pad 784 -> 7 full k-tiles of 128
KT1 = D_IN_PAD // 128
NJ = H // 128  # 8 feature tiles per hidden layer
BN_EPS = 1e-5
RSQRT_MAGIC = 0x5F3759DF

f32 = mybir.dt.float32
f32r = mybir.dt.float32r
i32 = mybir.dt.int32
bf16 = mybir.dt.bfloat16
AF = mybir.ActivationFunctionType
ALU = mybir.AluOpType

# t_vec scratch layout (free-dim float offsets, indexed by j 0..7)
V_S = 0       # BN scale per feature
V_T = 8       # BN shift per feature
V_M = 16      # mean
V_E2 = 24     # E[x^2]
V_VU = 32     # var + eps
V_R = 40      # rsqrt iterate
V_T1 = 48     # newton scratch
V_TMP = 56    # misc scratch
V_TMP2 = 64

# stats groups: list of (j_lo, j_hi) per layer, each >=10us apart so the
# serialized ~10us collective round trips never pile up
GROUPS_L1 = [(0, 2), (2, 4), (4, 6), (6, 8)]
GROUPS_H = [(0, 2), (2, 4), (4, 6), (6, 8)]


def build(nc):
    xT = nc.dram_tensor("xT", [KT1, NB, 128, 512], f32r, kind="ExternalInput")
    w1t = nc.dram_tensor("w1t", [D_IN_PAD, H], bf16, kind="ExternalInput")
    w2t = nc.dram_tensor("w2t", [H, H], bf16, kind="ExternalInput")
    w3t = nc.dram_tensor("w3t", [H, H], bf16, kind="ExternalInput")
    w4t = nc.dram_tensor("w4t", [H, D_OUT], bf16, kind="ExternalInput")
    gbp = nc.dram_tensor("gbp", [128, 6, 8], f32, kind="ExternalInput")
    outT = nc.dram_tensor("outT", [D_OUT, BPC], f32, kind="ExternalOutput")

    rg = [list(range(N_CORES))]

    with tile.TileContext(nc) as tc:
        with (
            tc.tile_pool(name="hp", bufs=2) as hpool,
            tc.tile_pool(name="wp", bufs=2) as wpool,
            tc.tile_pool(name="w4p", bufs=1) as w4pool,
            tc.tile_pool(name="stg", bufs=8) as stgpool,
            tc.tile_pool(name="outp", bufs=2) as outpool,
            tc.tile_pool(name="msc", bufs=1) as mpool,
            tc.tile_pool(name="ps", bufs=8, space="PSUM") as pspool,
            tc.tile_pool(name="dram", bufs=16, space="DRAM") as dpool,
        ):
            t_stats = mpool.tile([128, 192], f32, name="t_stats")
            t_part = mpool.tile([128, 16], f32, name="t_part")
            t_gst = mpool.tile([128, 16], f32, name="t_gst")
            t_vec = mpool.tile([128, 72], f32, name="t_vec")
            t_gb = mpool.tile([128, 48], f32, name="t_gb")
            t_const = mpool.tile([128, 48], f32, name="t_const")

            # single act-table load: force the sigmoid set (contains
            # sign/relu/sigmoid) before any other ScalarE op
            nc.scalar.activation(
                t_vec[:, V_TMP : V_TMP + 1], t_vec[:, V_TMP : V_TMP + 1], AF.Sigmoid
            )
            C_OFF = {"eighth": 0, "eps": 8, "one": 16, "magic": 24, "nhalf": 32, "3half": 40}
            nc.vector.memset(t_const[:, 0:8], 1.0 / N_CORES)
            nc.vector.memset(t_const[:, 8:16], BN_EPS)
            nc.vector.memset(t_const[:, 16:24].bitcast(i32), 1)
            nc.vector.memset(t_const[:, 24:32].bitcast(i32), RSQRT_MAGIC)
            nc.vector.memset(t_const[:, 32:40], -0.5)
            nc.vector.memset(t_const[:, 40:48], 1.5)

            # ---------------- input DMAs + weight sign prep ----------------
            xT_t = hpool.tile([128, KT1, NB, 512], f32r, name="xT_t", tag="h")
            W1s = wpool.tile([128, KT1, H], f32r, name="W1s", tag="w")
            W2s = wpool.tile([128, NJ, H], f32r, name="W2s", tag="w")
            W3s = wpool.tile([128, NJ, H], f32r, name="W3s", tag="w")
            W4s = w4pool.tile([128, NJ, D_OUT], f32r, name="W4s")

            def emit_w_dmas(wt_dram, nkt, tag):
                stgs = []
                for k in range(nkt):
                    for hh in range(2):
                        stg = stgpool.tile(
                            [128, H // 2], bf16, name=f"stg_{tag}_{k}_{hh}", tag="stg"
                        )
                        nc.sync.dma_start(
                            stg[:],
                            wt_dram[k * 128 : (k + 1) * 128, hh * 512 : (hh + 1) * 512],
                        )
                        stgs.append((k, hh, stg))
                return stgs

            def emit_w_signs(Wtile, stgs, after=None):
                first = [after]
                for k, hh, stg in stgs:
                    si = nc.scalar.activation(
                        Wtile[:, k, hh * 512 : (hh + 1) * 512], stg[:], AF.Sign
                    )
                    if first[0] is not None:
                        add_dep_helper(
                            si.ins, first[0], False,
                            "keep boundary ACT ops ahead of weight signs",
                        )
                        first[0] = None

            # --- warmup collective first on the gpsimd queue: its trigger is
            # ~1us of queue time and the ncfw first-call cost burns off in the
            # background while the input DMAs stream.
            with nc.named_scope("warmup_ar"):
                win = dpool.tile([128, 2], f32, name="warm_in")
                wout = dpool.tile([128, 2], f32, name="warm_out", addr_space="Shared")
                nc.gpsimd.collective_compute(
                    "AllReduce", ALU.add, replica_groups=rg,
                    ins=[win[:].opt()], outs=[wout[:].opt()],
                )
            # hot L1 inputs (w1, xb0, xb1) ride the gpsimd DMA queue, which
            # measures ~2x the sync queue's bandwidth; xb2/xb3 + the later
            # weights stream on sync in parallel.
            with nc.named_scope("w1prep"):
                # interleave per k: two w1 halves then the x tile
                for k in range(KT1):
                    for hh in range(2):
                        stg = stgpool.tile(
                            [128, H // 2], bf16, name=f"stg_w1_{k}_{hh}", tag="stg"
                        )
                        nc.gpsimd.dma_start(
                            stg[:],
                            w1t[k * 128 : (k + 1) * 128, hh * 512 : (hh + 1) * 512],
                        )
                        nc.scalar.activation(
                            W1s[:, k, hh * 512 : (hh + 1) * 512], stg[:], AF.Sign
                        )
                    nc.gpsimd.dma_start(xT_t[:, k, 0], xT[k, 0])
            with nc.named_scope("xload1"):
                for k in range(KT1):
                    nc.gpsimd.dma_start(xT_t[:, k, 1], xT[k, 1])
            with nc.named_scope("xload23"):
                for k in range(KT1):
                    nc.sync.dma_start(xT_t[:, k, 2], xT[k, 2])
                for k in range(KT1):
                    nc.sync.dma_start(xT_t[:, k, 3], xT[k, 3])
            with nc.named_scope("w2load"):
                w2_stgs = emit_w_dmas(w2t, NJ, "w2")
            nc.sync.dma_start(t_gb[:], gbp[:].rearrange("p a b -> p (a b)"))
            with nc.named_scope("w4prep"):
                stg4 = stgpool.tile([128, NJ, D_OUT], bf16, name="stg_w4", tag="stg")
                nc.sync.dma_start(
                    stg4[:], w4t[:].rearrange("(kt p) c -> p kt c", p=128)
                )
                nc.scalar.activation(
                    W4s[:].rearrange("p a b -> p (a b)"),
                    stg4[:].rearrange("p a b -> p (a b)"),
                    AF.Sign,
                )
            with nc.named_scope("w3load"):
                w3_stgs = emit_w_dmas(w3t, NJ, "w3")
            # w2 signs early (ScalarE idle during L1 until first applies)
            with nc.named_scope("w2sign"):
                emit_w_signs(W2s, w2_stgs)

            h1 = hpool.tile([128, NJ, NB, 512], f32r, name="h1", tag="h")
            h2 = hpool.tile([128, NJ, NB, 512], f32r, name="h2", tag="h")
            h3 = hpool.tile([128, NJ, NB, 512], f32r, name="h3", tag="h")

            # w3 signs are pinned after L1's last apply group so the scheduler
            # cannot hoist the sign burst into the L1->L2 boundary ACT window
            l1_last_apply = [None]

            # ---------------- helpers ----------------
            def drain_stats(out_h, j, b, with_stats=True):
                so = j * 24 + b * 6
                nc.vector.tensor_copy(out_h[:, j, b], cur_acc[(j, b)][:])
                if with_stats:
                    nc.vector.bn_stats(
                        t_stats[:, so : so + 6], out_h[:, j, b].bitcast(f32)
                    )

            cur_acc = {}

            def aggr_j(j):
                po = j * 2
                nc.vector.bn_aggr(
                    t_part[:, po : po + 2], t_stats[:, j * 24 : j * 24 + 24]
                )
                # rsqrt seed from the LOCAL variance (no collective wait):
                # rr[j] = bitcast(MAGIC - ((var_loc + eps)_int >> 1)); the
                # Newton steps in st_group then converge on the global var.
                tj = t_vec[:, V_T1 + j : V_T1 + j + 1]
                rj = t_vec[:, V_R + j : V_R + j + 1]
                nc.vector.tensor_scalar(
                    tj, t_part[:, po + 1 : po + 2], BN_EPS, None, op0=ALU.add
                )
                nc.vector.tensor_scalar(
                    tj.bitcast(i32), tj.bitcast(i32), 1, None,
                    op0=ALU.logical_shift_right,
                )
                nc.vector.tensor_tensor(
                    rj.bitcast(i32),
                    t_const[:, C_OFF["magic"] + j : C_OFF["magic"] + j + 1].bitcast(i32),
                    tj.bitcast(i32),
                    op=ALU.subtract,
                )
                # fold mean^2 into E2 slot: part[1] += part[0]^2
                nc.vector.tensor_tensor(
                    t_vec[:, V_TMP + j : V_TMP + j + 1],
                    t_part[:, po : po + 1],
                    t_part[:, po : po + 1],
                    op=ALU.mult,
                )
                nc.vector.tensor_tensor(
                    t_part[:, po + 1 : po + 2],
                    t_vec[:, V_TMP + j : V_TMP + j + 1],
                    t_part[:, po + 1 : po + 2],
                    op=ALU.add,
                )

            def collective_group(li, j_lo, j_hi):
                n = (j_hi - j_lo) * 2
                with nc.named_scope(f"L{li}_ar{j_lo}"):
                    cin = dpool.tile([128, n], f32, name=f"cin{li}_{j_lo}")
                    cout = dpool.tile(
                        [128, n], f32, name=f"cout{li}_{j_lo}", addr_space="Shared"
                    )
                    nc.gpsimd.dma_start(cin[:], t_part[:, j_lo * 2 : j_hi * 2])
                    nc.gpsimd.collective_compute(
                        "AllReduce", ALU.add, replica_groups=rg,
                        ins=[cin[:].opt()], outs=[cout[:].opt()],
                    )
                    nc.gpsimd.dma_start(t_gst[:, j_lo * 2 : j_hi * 2], cout[:])

            def st_group(li, j_lo, j_hi):
                """s = g*rsqrt(v+eps), t = b - m*s — entirely on GpSimd
                (tensor_tensor against constant tiles only, so the ops ride
                the same Pool queue as the collective and can never block
                VectorE drains). rsqrt via bit-trick init + 3 Newton steps."""
                with nc.named_scope(f"L{li}_st{j_lo}"):
                    gview = t_gst[:, j_lo * 2 : j_hi * 2].rearrange(
                        "p (j c) -> p j c", c=2
                    )
                    mm = t_vec[:, V_M + j_lo : V_M + j_hi]
                    e2 = t_vec[:, V_E2 + j_lo : V_E2 + j_hi]
                    vu = t_vec[:, V_VU + j_lo : V_VU + j_hi]
                    rr = t_vec[:, V_R + j_lo : V_R + j_hi]
                    t1 = t_vec[:, V_T1 + j_lo : V_T1 + j_hi]
                    tp2 = t_vec[:, V_TMP2 + j_lo : V_TMP2 + j_hi]
                    sv = t_vec[:, V_S + j_lo : V_S + j_hi]
                    tv = t_vec[:, V_T + j_lo : V_T + j_hi]
                    cs = lambda name: t_const[:, C_OFF[name] + j_lo : C_OFF[name] + j_hi]
                    g_sl = t_gb[:, (li - 1) * 16 + j_lo : (li - 1) * 16 + j_hi]
                    b_sl = t_gb[:, (li - 1) * 16 + 8 + j_lo : (li - 1) * 16 + 8 + j_hi]
                    tt = nc.gpsimd.tensor_tensor
                    tt(mm, gview[:, :, 0], cs("eighth"), op=ALU.mult)
                    tt(e2, gview[:, :, 1], cs("eighth"), op=ALU.mult)
                    tt(tp2, mm, mm, op=ALU.mult)
                    tt(vu, e2, tp2, op=ALU.subtract)
                    tt(vu, vu, cs("eps"), op=ALU.add)
                    # rr was seeded in aggr_j from the local variance
                    for _ in range(3):
                        tt(t1, rr, rr, op=ALU.mult)
                        tt(t1, t1, vu, op=ALU.mult)
                        tt(t1, t1, cs("nhalf"), op=ALU.mult)
                        tt(t1, t1, cs("3half"), op=ALU.add)
                        tt(rr, rr, t1, op=ALU.mult)
                    tt(sv, g_sl, rr, op=ALU.mult)
                    tt(tp2, mm, sv, op=ALU.mult)
                    last = tt(tv, b_sl, tp2, op=ALU.subtract)
                    return last

            def apply_group(li, out_h, j_lo, j_hi):
                """BN scale/shift + ReLU in place on ScalarE; one 2048-wide
                ACT per feature tile (all 4 batch tiles at once)."""
                last = None
                with nc.named_scope(f"L{li}_apply{j_lo}"):
                    for j in range(j_lo, j_hi):
                        last = nc.scalar.activation(
                            out_h[:, j].rearrange("p b f -> p (b f)"),
                            out_h[:, j].rearrange("p b f -> p (b f)").bitcast(f32),
                            AF.Relu,
                            bias=t_vec[:, V_T + j : V_T + j + 1],
                            scale=t_vec[:, V_S + j : V_S + j + 1],
                        )
                return last

            # ---------------- layer 1: x -> h1 ----------------
            with nc.named_scope("L1_sweepA"):
                # b0 as a k-outer sweep (chases the streaming x/w1 DMA)
                accs = [
                    pspool.tile([128, 512], f32, name=f"ps1_s0_j{j}", tag="ps")
                    for j in range(NJ)
                ]
                for k in range(KT1):
                    for j in range(NJ):
                        nc.tensor.matmul(
                            accs[j][:],
                            W1s[:, k, j * 128 : (j + 1) * 128],
                            xT_t[:, k, 0],
                            start=(k == 0),
                            stop=(k == KT1 - 1),
                        )
                for j in range(NJ):
                    cur_acc[(j, 0)] = accs[j]
                    drain_stats(h1, j, 0)
            with nc.named_scope("L1_phaseB"):
                # (b1,b2,b3) triples per j so early j-tiles finish stats
                # early. j4/j5 are produced before j2/j3 so the {4,5} group's
                # collective launches ~11us earlier: its applies are the last
                # ones L2's opening partial blocks consume (k order 0..5).
                for j, grp in [(0, None), (1, (0, 2)), (4, None), (5, (4, 6)),
                               (2, None), (3, (2, 4)), (6, None), (7, (6, 8))]:
                    for b in (1, 2, 3):
                        acc = pspool.tile(
                            [128, 512], f32, name=f"ps1_p_j{j}_b{b}", tag="ps"
                        )
                        for k in range(KT1):
                            nc.tensor.matmul(
                                acc[:],
                                W1s[:, k, j * 128 : (j + 1) * 128],
                                xT_t[:, k, b],
                                start=(k == 0),
                                stop=(k == KT1 - 1),
                            )
                        cur_acc[(j, b)] = acc
                        drain_stats(h1, j, b)
                    aggr_j(j)
                    if grp is not None:
                        collective_group(1, *grp)
                        st_group(1, *grp)
                        la = apply_group(1, h1, *grp)
                        l1_last_apply[0] = la.ins

            def hidden_layer(li, Wcur, rhs, out_h, post_fix=None):
                """L2/L3: j0/j1/j2 contract k0..5; the k6,k7 contribution is
                patched in right after j2 (PSUM matmuls + Pool adds), by which
                point the previous layer's {6,7} applies have landed."""
                gi = [0]
                groups = GROUPS_H

                def maybe_group(j):
                    if gi[0] < len(groups) and j == groups[gi[0]][1] - 1:
                        g = groups[gi[0]]
                        collective_group(li, *g)
                        st_group(li, *g)
                        apply_group(li, out_h, *g)
                        gi[0] += 1

                def jblock(j, klo, khi):
                    with nc.named_scope(f"L{li}_j{j}"):
                        for b in range(NB):
                            acc = pspool.tile(
                                [128, 512], f32, name=f"ps{li}_j{j}_b{b}", tag="ps"
                            )
                            for k in range(klo, khi):
                                nc.tensor.matmul(
                                    acc[:],
                                    Wcur[:, k, j * 128 : (j + 1) * 128],
                                    rhs[:, k, b],
                                    start=(k == klo),
                                    stop=(k == khi - 1),
                                )
                            cur_acc[(j, b)] = acc
                            drain_stats(out_h, j, b, with_stats=(klo == 0 and khi == NJ))

                NFIX = 3
                for j in range(NFIX):
                    jblock(j, 0, NJ - 2)
                # fixup: k6,k7 for j0..j2 via PSUM + Pool add, then stats.
                # All adds/stats run before any group launch so none of them
                # queue behind a blocking collective on the Pool FIFO.
                with nc.named_scope(f"L{li}_fix"):
                    for j in range(NFIX):
                        for b in range(NB):
                            facc = pspool.tile(
                                [128, 512], f32, name=f"psf{li}_j{j}_b{b}", tag="ps"
                            )
                            for k in (NJ - 2, NJ - 1):
                                nc.tensor.matmul(
                                    facc[:],
                                    Wcur[:, k, j * 128 : (j + 1) * 128],
                                    rhs[:, k, b],
                                    start=(k == NJ - 2),
                                    stop=(k == NJ - 1),
                                )
                            nc.vector.tensor_tensor(
                                out_h[:, j, b],
                                out_h[:, j, b].bitcast(f32),
                                facc[:],
                                op=ALU.add,
                            )
                            so = j * 24 + b * 6
                            nc.vector.bn_stats(
                                t_stats[:, so : so + 6], out_h[:, j, b].bitcast(f32)
                            )
                        aggr_j(j)
                    collective_group(li, *groups[0])
                    st_group(li, *groups[0])
                    apply_group(li, out_h, *groups[0])
                    gi[0] = 1
                if post_fix is not None:
                    post_fix()
                for j in range(NFIX, NJ):
                    jblock(j, 0, NJ)
                    aggr_j(j)
                    maybe_group(j)

            def post_fix_w3signs():
                with nc.named_scope("w3sign"):
                    emit_w_signs(W3s, w3_stgs, after=l1_last_apply[0])

            hidden_layer(2, W2s, h1, h2, post_fix=post_fix_w3signs)
            hidden_layer(3, W3s, h2, h3)

            # ---------------- head: h3 -> sigmoid out ----------------
            with nc.named_scope("L4"):
                haccs = []
                for b in range(NB):
                    acc = pspool.tile([D_OUT, 512], f32, name=f"ps4_b{b}", tag="ps")
                    for k in range(NJ - 2):
                        nc.tensor.matmul(
                            acc[:], W4s[:, k], h3[:, k, b],
                            start=(k == 0), stop=False,
                        )
                    haccs.append(acc)
                for b in range(NB):
                    nc.tensor.matmul(
                        haccs[b][:], W4s[:, NJ - 2], h3[:, NJ - 2, b],
                        start=False, stop=False,
                    )
                    nc.tensor.matmul(
                        haccs[b][:], W4s[:, NJ - 1], h3[:, NJ - 1, b],
                        start=False, stop=True,
                    )
                    osb = outpool.tile([D_OUT, 512], f32, name=f"osb{b}", tag="osb")
                    nc.scalar.activation(osb[:], haccs[b][:], AF.Sigmoid)
                    nc.sync.dma_start(outT[:, b * 512 : (b + 1) * 512], osb[:])

    nc.compile()
    return nc


_NC = None
_LAST_RESULTS = None


def _get_nc():
    global _NC
    if _NC is None:
        nc = bacc.Bacc(
            "TRN2", target_bir_lowering=False, debug=False, num_devices=N_CORES
        )
        build(nc)
        _NC = nc
    return _NC


def kernel(**inputs):
    x = np.ascontiguousarray(inputs["x"], dtype=np.float32)
    w1 = np.asarray(inputs["w1"], dtype=np.float32)
    w2 = np.asarray(inputs["w2"], dtype=np.float32)
    w3 = np.asarray(inputs["w3"], dtype=np.float32)
    w4 = np.asarray(inputs["w4"], dtype=np.float32)
    gb = np.stack(
        [
            np.asarray(inputs[n], dtype=np.float32)
            for n in ("g1", "b1", "g2", "b2", "g3", "b3")
        ]
    )  # [6, 1024]

    import ml_dtypes

    bf = ml_dtypes.bfloat16
    w1tp = np.zeros((D_IN_PAD, H), bf)
    w1tp[:D_IN] = w1.T.astype(bf)
    w2tp = np.ascontiguousarray(w2.T.astype(bf))
    w3tp = np.ascontiguousarray(w3.T.astype(bf))
    w4tp = np.ascontiguousarray(w4.T.astype(bf))
    gbp = np.ascontiguousarray(gb.reshape(6, 8, 128).transpose(2, 0, 1))  # [128,6,8]

    nc = _get_nc()
    in_maps = []
    for c in range(N_CORES):
        xs = np.zeros((D_IN_PAD, BPC), np.float32)
        xs[:D_IN] = x[c * BPC : (c + 1) * BPC].T
        xs = np.ascontiguousarray(
            xs.reshape(KT1, 128, NB, 512).transpose(0, 2, 1, 3)
        )
        in_maps.append(
            {"xT": xs, "w1t": w1tp, "w2t": w2tp, "w3t": w3tp, "w4t": w4tp, "gbp": gbp}
        )

    last_err = None
    for _attempt in range(3):
        try:
            res = run_bass_kernel_spmd(nc, in_maps, core_ids=list(range(N_CORES)))
            break
        except Exception as e:  # transient NRT_EXEC_UNIT_UNRECOVERABLE etc.
            last_err = e
    else:
        raise last_err
    global _LAST_RESULTS
    _LAST_RESULTS = res
    out = np.empty((B_TOT, D_OUT), dtype=np.float32)
    for c in range(N_CORES):
        out[c * BPC : (c + 1) * BPC] = res.results[c]["outT"].T
    return out


# revision 26
# speedup vs baseline: 1.2640x; 1.0998x over previous
"""Binarized MLP forward (BinaryConnect, training-mode BatchNorm) on 8 TRN2 cores.

Strategy: data-parallel over the batch (16384 -> 8 x 2048), weights replicated.
All activations kept TRANSPOSED on device ([features, batch]) so that
 - matmuls use binarized weights as the stationary operand,
 - BatchNorm stats are free-axis reductions (bn_stats on VectorE),
 - BN apply + ReLU is a single per-partition scale/bias activation on ScalarE.
Per-feature batch statistics are all-reduced across the 8 cores (8 KB/layer),
split into an early group (features 0..895, overlapped with the layer tail)
and a late group (last 128 features) to keep the boundary short.
Matmuls run in float32r (full PE rate at N=512; binarized +-1 weights exact).
"""
import os
import numpy as np

import concourse.bass as bass
import concourse.bacc as bacc
import concourse.tile as tile
from concourse.tile_rust import add_dep_helper
import concourse.mybir as mybir
from concourse.bass_utils import run_bass_kernel_spmd

N_CORES = 8
B_TOT = 16384
BPC = B_TOT // N_CORES  # 2048 batch rows per core
NB = BPC // 512  # 4 free-dim tiles of 512
D_IN, H, D_OUT = 784, 1024, 10
D_IN_PAD = 896  # pad 784 -> 7 full k-tiles of 128
KT1 = D_IN_PAD // 128
NJ = H // 128  # 8 feature tiles per hidden layer
BN_EPS = 1e-5

f32 = mybir.dt.float32
f32r = mybir.dt.float32r
i32 = mybir.dt.int32
bf16 = mybir.dt.bfloat16
AF = mybir.ActivationFunctionType
ALU = mybir.AluOpType

# t_vec scratch layout (free-dim float offsets)
V_S = 0       # BN scale per feature (8)
V_T = 8       # BN shift per feature (8)
V_M = 16      # mean
V_E2 = 24
V_VU = 32     # var, then var+eps
V_SQ = 40     # sqrt(var+eps)
V_R = 48      # rsqrt
V_TMP = 56
V_TMP2 = 64


def build(nc):
    xT = nc.dram_tensor("xT", [KT1, NB, 128, 512], f32r, kind="ExternalInput")
    w1t = nc.dram_tensor("w1t", [D_IN_PAD, H], bf16, kind="ExternalInput")
    w2t = nc.dram_tensor("w2t", [H, H], bf16, kind="ExternalInput")
    w3t = nc.dram_tensor("w3t", [H, H], bf16, kind="ExternalInput")
    w4t = nc.dram_tensor("w4t", [H, D_OUT], bf16, kind="ExternalInput")
    gbp = nc.dram_tensor("gbp", [128, 6, 8], f32, kind="ExternalInput")
    outT = nc.dram_tensor("outT", [D_OUT, BPC], f32, kind="ExternalOutput")

    rg = [list(range(N_CORES))]

    with tile.TileContext(nc) as tc:
        with (
            tc.tile_pool(name="hp", bufs=2) as hpool,
            tc.tile_pool(name="wp", bufs=2) as wpool,
            tc.tile_pool(name="w4p", bufs=1) as w4pool,
            tc.tile_pool(name="stg", bufs=6) as stgpool,
            tc.tile_pool(name="outp", bufs=2) as outpool,
            tc.tile_pool(name="msc", bufs=1) as mpool,
            tc.tile_pool(name="ps", bufs=8, space="PSUM") as pspool,
            tc.tile_pool(name="dram", bufs=1, space="DRAM") as dpool,
        ):
            t_stats = mpool.tile([128, 192], f32, name="t_stats")
            t_part = mpool.tile([128, 16], f32, name="t_part")
            t_gst = mpool.tile([128, 16], f32, name="t_gst")
            t_vec = mpool.tile([128, 72], f32, name="t_vec")
            t_gb = mpool.tile([128, 48], f32, name="t_gb")

            # --- warmup collective: absorb first-call ncfw/algorithm cost.
            # Pure DRAM->DRAM with unread output: zero coupling with the
            # compute DMA queues or SBUF dependency tracking.
            with nc.named_scope("warmup_ar"):
                win = dpool.tile([128, 2], f32, name="warm_in")
                wout = dpool.tile([128, 2], f32, name="warm_out", addr_space="Shared")
                nc.gpsimd.collective_compute(
                    "AllReduce", ALU.add, replica_groups=rg,
                    ins=[win[:].opt()], outs=[wout[:].opt()],
                )

            def prep_w(wt_dram, Wtile, nkt, tag_suffix, after=None):
                """DMA raw transposed weights into staging (half-tiles for a
                finer DMA/Sign pipeline), binarize (Sign). The first Sign is
                order-pinned after `after` so the scheduler cannot hoist the
                sign burst into the previous layer's boundary ACT window."""
                after_inst = [after]
                for k in range(nkt):
                    for hh in range(2):
                        stg = stgpool.tile(
                            [128, H // 2], bf16, name=f"stg_{tag_suffix}_{k}_{hh}", tag="stg"
                        )
                        nc.sync.dma_start(
                            stg[:],
                            wt_dram[k * 128 : (k + 1) * 128, hh * 512 : (hh + 1) * 512],
                        )
                        si = nc.scalar.activation(
                            Wtile[:, k, hh * 512 : (hh + 1) * 512], stg[:], AF.Sign
                        )
                        if after_inst[0] is not None:
                            add_dep_helper(
                                si.ins, after_inst[0], False,
                                "keep boundary ACT ops ahead of weight signs",
                            )
                            after_inst[0] = None

            # --- input loads, in first-consumer order: the layer-1 j0 column
            # needs W1s[k] and xT[b=0, k] for every k first.
            xT_t = hpool.tile([128, KT1, NB, 512], f32r, name="xT_t", tag="h")
            W1s = wpool.tile([128, KT1, H], f32r, name="W1s", tag="w")
            with nc.named_scope("w1prep"):
                for k in range(KT1):
                    for hh in range(2):
                        stg = stgpool.tile(
                            [128, H // 2], bf16, name=f"stg_w1_{k}_{hh}", tag="stg"
                        )
                        nc.sync.dma_start(
                            stg[:],
                            w1t[k * 128 : (k + 1) * 128, hh * 512 : (hh + 1) * 512],
                        )
                        nc.scalar.activation(
                            W1s[:, k, hh * 512 : (hh + 1) * 512], stg[:], AF.Sign
                        )
                    nc.sync.dma_start(xT_t[:, k, 0], xT[k, 0])
            with nc.named_scope("xload"):
                # b1..b3 on the gpsimd DMA queue (~2x sync bandwidth): the w2
                # staging DMAs behind them on sync then land mid-L1 instead of
                # at the L1->L2 boundary
                for b in range(1, NB):
                    for k in range(KT1):
                        nc.gpsimd.dma_start(xT_t[:, k, b], xT[k, b])
            nc.sync.dma_start(t_gb[:], gbp[:].rearrange("p a b -> p (a b)"))

            W4s = w4pool.tile([128, NJ, D_OUT], f32r, name="W4s")
            with nc.named_scope("w4prep"):
                stg4 = stgpool.tile([128, NJ, D_OUT], bf16, name="stg_w4", tag="stg")
                nc.sync.dma_start(
                    stg4[:], w4t[:].rearrange("(kt p) c -> p kt c", p=128)
                )
                nc.scalar.activation(
                    W4s[:].rearrange("p a b -> p (a b)"),
                    stg4[:].rearrange("p a b -> p (a b)"),
                    AF.Sign,
                )

            h1 = hpool.tile([128, NJ, NB, 512], f32r, name="h1", tag="h")
            h2 = hpool.tile([128, NJ, NB, 512], f32r, name="h2", tag="h")
            h3 = hpool.tile([128, NJ, NB, 512], f32r, name="h3", tag="h")

            def collective_group(li, j_lo, j_hi, gtag):
                """All-reduce partial stats for feature tiles [j_lo, j_hi)."""
                n = (j_hi - j_lo) * 2
                with nc.named_scope(f"L{li}_ar{gtag}"):
                    cin = dpool.tile([128, n], f32, name=f"cin{li}{gtag}")
                    cout = dpool.tile(
                        [128, n], f32, name=f"cout{li}{gtag}", addr_space="Shared"
                    )
                    nc.gpsimd.dma_start(
                        cin[:], t_part[:, j_lo * 2 : j_hi * 2]
                    )
                    nc.gpsimd.collective_compute(
                        "AllReduce", ALU.add, replica_groups=rg,
                        ins=[cin[:].opt()], outs=[cout[:].opt()],
                    )
                    nc.gpsimd.dma_start(t_gst[:, j_lo * 2 : j_hi * 2], cout[:])

            def st_group(li, j_lo, j_hi, gtag):
                """s = g*rsqrt(v+eps), t = b - m*s for feature tiles [j_lo, j_hi)."""
                with nc.named_scope(f"L{li}_st{gtag}"):
                    gview = t_gst[:, j_lo * 2 : j_hi * 2].rearrange(
                        "p (j c) -> p j c", c=2
                    )
                    mm = t_vec[:, V_M + j_lo : V_M + j_hi]
                    e2 = t_vec[:, V_E2 + j_lo : V_E2 + j_hi]
                    vu = t_vec[:, V_VU + j_lo : V_VU + j_hi]
                    sq = t_vec[:, V_SQ + j_lo : V_SQ + j_hi]
                    rr = t_vec[:, V_R + j_lo : V_R + j_hi]
                    tp2 = t_vec[:, V_TMP2 + j_lo : V_TMP2 + j_hi]
                    sv = t_vec[:, V_S + j_lo : V_S + j_hi]
                    tv = t_vec[:, V_T + j_lo : V_T + j_hi]
                    g_sl = t_gb[:, (li - 1) * 16 + j_lo : (li - 1) * 16 + j_hi]
                    b_sl = t_gb[:, (li - 1) * 16 + 8 + j_lo : (li - 1) * 16 + 8 + j_hi]
                    nc.vector.tensor_scalar(mm, gview[:, :, 0], 1.0 / N_CORES, None, op0=ALU.mult)
                    nc.vector.tensor_scalar(e2, gview[:, :, 1], 1.0 / N_CORES, None, op0=ALU.mult)
                    nc.vector.tensor_tensor(tp2, mm, mm, op=ALU.mult)
                    nc.vector.tensor_tensor(vu, e2, tp2, op=ALU.subtract)
                    nc.vector.tensor_scalar(vu, vu, BN_EPS, None, op0=ALU.add)
                    sq_inst = nc.scalar.activation(sq, vu, AF.Sqrt)
                    nc.vector.reciprocal(rr, sq)
                    nc.vector.tensor_tensor(sv, g_sl, rr, op=ALU.mult)
                    nc.vector.tensor_tensor(tp2, mm, sv, op=ALU.mult)
                    nc.vector.tensor_tensor(tv, b_sl, tp2, op=ALU.subtract)
                    return sq_inst

            def apply_group(li, out_h, j_lo, j_hi):
                last = None
                with nc.named_scope(f"L{li}_apply{j_lo}"):
                    for j in range(j_lo, j_hi):
                        for b in range(NB):
                            last = nc.scalar.activation(
                                out_h[:, j, b],
                                out_h[:, j, b].bitcast(f32),
                                AF.Relu,
                                bias=t_vec[:, V_T + j : V_T + j + 1],
                                scale=t_vec[:, V_S + j : V_S + j + 1],
                            )
                return last

            def layer(li, Wcur, nkt, rhs, out_h, prep_next, split_b0=False):
                """One hidden layer: matmuls + stats + allreduce + BN/ReLU apply."""
                if split_b0:
                    # b0-only warm pass: dense PE work on the first-arriving
                    # rhs chunk while the rest of the input streams in
                    with nc.named_scope(f"L{li}_mm0"):
                        for j in range(NJ):
                            acc = pspool.tile(
                                [128, 512], f32, name=f"ps_l{li}p0_j{j}", tag="ps"
                            )
                            for k in range(nkt):
                                nc.tensor.matmul(
                                    acc[:],
                                    Wcur[:, k, j * 128 : (j + 1) * 128],
                                    rhs[:, k, 0],
                                    start=(k == 0),
                                    stop=(k == nkt - 1),
                                )
                            nc.vector.tensor_copy(out_h[:, j, 0], acc[:])
                b_lo = 1 if split_b0 else 0
                with nc.named_scope(f"L{li}_mm"):
                    for j in range(NJ):
                        accs = [
                            pspool.tile(
                                [128, 512], f32, name=f"ps_l{li}_j{j}_b{b}", tag="ps"
                            )
                            for b in range(b_lo, NB)
                        ]
                        for b in range(b_lo, NB):
                            for k in range(nkt):
                                nc.tensor.matmul(
                                    accs[b - b_lo][:],
                                    Wcur[:, k, j * 128 : (j + 1) * 128],
                                    rhs[:, k, b],
                                    start=(k == 0),
                                    stop=(k == nkt - 1),
                                )
                        for b in range(b_lo, NB):
                            nc.vector.tensor_copy(out_h[:, j, b], accs[b - b_lo][:])
                        for b in range(NB):
                            so = j * 24 + b * 6
                            nc.vector.bn_stats(
                                t_stats[:, so : so + 6], out_h[:, j, b].bitcast(f32)
                            )
                        # per-j partial: bn_aggr -> (mean, var); then E2 = var + mean^2
                        po = j * 2
                        nc.vector.bn_aggr(
                            t_part[:, po : po + 2],
                            t_stats[:, j * 24 : j * 24 + 24],
                        )
                        nc.vector.tensor_tensor(
                            t_vec[:, V_TMP + j : V_TMP + j + 1],
                            t_part[:, po : po + 1],
                            t_part[:, po : po + 1],
                            op=ALU.mult,
                        )
                        nc.vector.tensor_tensor(
                            t_part[:, po + 1 : po + 2],
                            t_vec[:, V_TMP + j : V_TMP + j + 1],
                            t_part[:, po + 1 : po + 2],
                            op=ALU.add,
                        )
                        if j == 1 and prep_next is not None:
                            prep_next(prev_apply[0])
                        if j == NJ - 3:
                            # early group: all-reduce features (lo)..(NJ-2);
                            # finishes during the j6/j7 tail
                            collective_group(li, 0, NJ - 2, "a")
                        if j == NJ - 2:
                            # emitted before j7's drains so the DVE stream
                            # does the s,t math as soon as the data is back
                            st_group(li, 0, NJ - 2, "a")
                            apply_a = apply_group(li, out_h, 0, NJ - 2)
                # late group: the last two feature tiles
                collective_group(li, NJ - 2, NJ, "b")
                sq_b = st_group(li, NJ - 2, NJ, "b")
                if apply_a is not None and sq_b is not None:
                    add_dep_helper(
                        sq_b.ins, apply_a.ins, False,
                        "group-a applies precede group-b sqrt on ScalarE",
                    )
                return apply_group(li, out_h, NJ - 2, NJ)

            W2s = wpool.tile([128, NJ, H], f32r, name="W2s", tag="w")
            W3s = wpool.tile([128, NJ, H], f32r, name="W3s", tag="w")

            prev_apply = [None]
            a1 = layer(1, W1s, KT1, xT_t, h1,
                       lambda after: prep_w(w2t, W2s, NJ, "w2", after),
                       split_b0=True)
            prev_apply[0] = a1.ins if a1 is not None else None
            a2 = layer(2, W2s, NJ, h1, h2,
                       lambda after: prep_w(w3t, W3s, NJ, "w3", after))
            prev_apply[0] = a2.ins if a2 is not None else None
            layer(3, W3s, NJ, h2, h3, None)

            # ---- head: 10-wide binarized linear + sigmoid ------------------
            with nc.named_scope("L4"):
                for b in range(NB):
                    acc = pspool.tile([D_OUT, 512], f32, name=f"ps_l4_b{b}", tag="ps")
                    for k in range(NJ):
                        nc.tensor.matmul(
                            acc[:],
                            W4s[:, k],
                            h3[:, k, b],
                            start=(k == 0),
                            stop=(k == NJ - 1),
                        )
                    osb = outpool.tile([D_OUT, 512], f32, name=f"osb{b}", tag="osb")
                    nc.scalar.activation(osb[:], acc[:], AF.Sigmoid)
                    nc.sync.dma_start(outT[:, b * 512 : (b + 1) * 512], osb[:])

    nc.compile()
    return nc


_NC = None
_LAST_RESULTS = None


def _get_nc():
    global _NC
    if _NC is None:
        nc = bacc.Bacc(
            "TRN2", target_bir_lowering=False, debug=False, num_devices=N_CORES
        )
        build(nc)
        _NC = nc
    return _NC


def kernel(**inputs):
    x = np.ascontiguousarray(inputs["x"], dtype=np.float32)
    w1 = np.asarray(inputs["w1"], dtype=np.float32)
    w2 = np.asarray(inputs["w2"], dtype=np.float32)
    w3 = np.asarray(inputs["w3"], dtype=np.float32)
    w4 = np.asarray(inputs["w4"], dtype=np.float32)
    gb = np.stack(
        [
            np.asarray(inputs[n], dtype=np.float32)
            for n in ("g1", "b1", "g2", "b2", "g3", "b3")
        ]
    )  # [6, 1024]

    import ml_dtypes

    bf = ml_dtypes.bfloat16
    w1t = np.zeros((D_IN_PAD, H), bf)
    w1t[:D_IN] = w1.T.astype(bf)
    w2t = np.ascontiguousarray(w2.T.astype(bf))
    w3t = np.ascontiguousarray(w3.T.astype(bf))
    w4t = np.ascontiguousarray(w4.T.astype(bf))
    gbp = np.ascontiguousarray(gb.reshape(6, 8, 128).transpose(2, 0, 1))  # [128,6,8]

    nc = _get_nc()
    in_maps = []
    for c in range(N_CORES):
        xs = np.zeros((D_IN_PAD, BPC), np.float32)
        xs[:D_IN] = x[c * BPC : (c + 1) * BPC].T
        xs = np.ascontiguousarray(
            xs.reshape(KT1, 128, NB, 512).transpose(0, 2, 1, 3)
        )
        in_maps.append(
            {"xT": xs, "w1t": w1t, "w2t": w2t, "w3t": w3t, "w4t": w4t, "gbp": gbp}
        )

    last_err = None
    for _attempt in range(3):
        try:
            res = run_bass_kernel_spmd(nc, in_maps, core_ids=list(range(N_CORES)))
            break
        except Exception as e:  # transient NRT_EXEC_UNIT_UNRECOVERABLE etc.
            last_err = e
    else:
        raise last_err
    global _LAST_RESULTS
    _LAST_RESULTS = res
    out = np.empty((B_TOT, D_OUT), dtype=np.float32)
    for c in range(N_CORES):
        out[c * BPC : (c + 1) * BPC] = res.results[c]["outT"].T
    return out

